# revision 37
# baseline (speedup 1.0000x reference)
"""TopK sparse-autoencoder forward on 8 Trainium2 NeuronCores.

Strategy (tensor-parallel over the G=24576 feature dim, 3072 per core):

  Launch 1 (per core): LayerNorm(x) (all rows, replicated work),
    preact slice = xn @ W_enc[:, slice] in exact fp32 on the PE,
    per-512-chunk top-16 candidates per row (vector.max + match_replace),
    preact slice + candidates + (mu, std) -> HBM.

  Host: exact per-row 64th-largest threshold t64 from the merged
    candidates (np.partition), with a sufficiency check per chunk and an
    exact fallback from the full preact slices for any flagged row.

  Launch 2 (per core): feats slice = relu(preact) * (preact >= t64),
    per-feature fire-sums (colsum) for the dead mask, feats^T tiles
    (PE transpose, rounded to float32r) staged via HBM, then the decode
    partial product feats_slice @ W_dec[slice, :] in float32r.

  Host: out = (sum of partials + b_dec) * std + mu; dead mask from the
    colsums (for this problem's data no feature is dead, so the aux path
    degenerates to dead_recon = mu and num_dead = 0; a full numpy
    fallback handles the general case).
"""
import os
import numpy as np
from contextlib import ExitStack

import concourse.bass as bass
import concourse.mybir as mybir
from concourse import tile
from concourse.bass_utils import run_bass_kernel_spmd


def _run_spmd(nc, in_maps, core_ids, tries=3):
    """The axon-tunneled device occasionally hard-faults transiently
    (NRT_EXEC_UNIT_UNRECOVERABLE); retry a couple of times."""
    import time as _time
    last = None
    for i in range(tries):
        try:
            return run_bass_kernel_spmd(nc, in_maps, core_ids)
        except Exception as e:   # noqa: BLE001
            last = e
            _time.sleep(5.0 * (i + 1))
    raise last

dt = mybir.dt
alu = mybir.AluOpType
act_fn = mybir.ActivationFunctionType

B, D, G, O = 8192, 768, 24576, 3072
NCORES = 8
GS = G // NCORES            # 3072 features per core
EPS = 1e-5
P = 128
NRT = B // P                # 64 row tiles
DT = D // P                 # 6 d-tiles
NGC = GS // 512             # 6 g-chunks of 512 per core slice
TOPK = 64
CHUNK_CAND = 16             # top-16 kept per 512-chunk


def split_waits(nc, max_waits: int = 1):
    """This walrus build accepts only one sync-wait command per
    instruction; move excess waits onto preceding same-engine NoOps."""
    ctr = 0
    for f in nc.m.functions:
        for b in f.blocks:
            insns = b.instructions
            out = []
            changed = False
            for inst in insns:
                si = inst.sync_info
                if si is not None and si.on_wait and len(si.on_wait) > max_waits:
                    waits = list(si.on_wait)
                    for w in waits[:-max_waits]:
                        ctr += 1
                        nop = mybir.InstNoOp(
                            name=f"wait-split-{ctr}", ins=[], outs=[],
                            engine=inst.engine,
                        )
                        nop.sync_info = mybir.SyncInfo(on_wait=[w], on_update=[])
                        out.append(nop)
                    inst.sync_info = mybir.SyncInfo(
                        on_wait=waits[-max_waits:], on_update=list(si.on_update)
                    )
                    changed = True
                out.append(inst)
            if changed:
                b.instructions = out
    return ctr


def build_l1():
    """LN + exact-precision encode via 3-term float32r split:
    xn @ W ≈ xr@Wr + xe@Wr + xr@We with xr/Wr rounded to float32r and
    xe/We the (rounded) residuals — max abs err ~2e-7, at 3 cycles/row
    instead of fp32's 4."""
    nc = bass.Bass(trn_type="TRN2")
    x = nc.dram_tensor("x", [B, D], dt.float32, kind="ExternalInput")
    wencr = nc.dram_tensor("wencr", [D, GS], dt.float32, kind="ExternalInput")
    wence = nc.dram_tensor("wence", [D, GS], dt.float32, kind="ExternalInput")
    eye = nc.dram_tensor("eye", [P, P], dt.float32, kind="ExternalInput")
    preact = nc.dram_tensor("preact", [B, GS], dt.float32, kind="ExternalOutput")
    cands = nc.dram_tensor("cands", [B, NGC * CHUNK_CAND], dt.float32,
                           kind="ExternalOutput")
    mustd = nc.dram_tensor("mustd", [B, 2], dt.float32, kind="ExternalOutput")

    with tile.TileContext(nc) as tc, ExitStack() as ctx:
        const = ctx.enter_context(tc.tile_pool(name="const", bufs=1))
        wpool = ctx.enter_context(tc.tile_pool(name="wpool", bufs=1))
        sb = ctx.enter_context(tc.tile_pool(name="sb", bufs=2))
        small = ctx.enter_context(tc.tile_pool(name="small", bufs=4))
        pst = ctx.enter_context(tc.tile_pool(name="pst", bufs=2, space="PSUM"))
        pse = ctx.enter_context(tc.tile_pool(name="pse", bufs=3, space="PSUM"))

        ident = const.tile([P, P], dt.float32r, tag="ident")
        nc.sync.dma_start(out=ident[:], in_=eye[:].bitcast(dt.float32r))

        # resident W_enc slice splits: block t holds rows [t*128,(t+1)*128)
        wr = wpool.tile([P, DT, GS], dt.float32r, tag="wr")
        we = wpool.tile([P, DT, GS], dt.float32r, tag="we")
        for t in range(DT):
            nc.sync.dma_start(out=wr[:, t, :],
                              in_=wencr[t * P:(t + 1) * P, :].bitcast(dt.float32r))
            nc.sync.dma_start(out=we[:, t, :],
                              in_=wence[t * P:(t + 1) * P, :].bitcast(dt.float32r))

        for rt in range(NRT):
            r0 = rt * P
            xt = sb.tile([P, D], dt.float32, tag="xt")
            nc.sync.dma_start(out=xt[:], in_=x[r0:r0 + P, :])

            # ---- LayerNorm stats ----
            s1 = small.tile([P, 1], dt.float32, tag="s1")
            nc.vector.tensor_reduce(out=s1[:], in_=xt[:],
                                    axis=mybir.AxisListType.XYZW, op=alu.add)
            xn = sb.tile([P, D], dt.float32, tag="xn")
            ssq = small.tile([P, 1], dt.float32, tag="ssq")
            # squares land in xn's buffer (overwritten right after)
            nc.scalar.activation(out=xn[:], in_=xt[:], func=act_fn.Square,
                                 accum_out=ssq[:])
            mu = small.tile([P, 1], dt.float32, tag="mu")
            nc.vector.tensor_scalar_mul(out=mu[:], in0=s1[:], scalar1=1.0 / D)
            t1 = small.tile([P, 1], dt.float32, tag="t1")
            nc.vector.tensor_tensor(out=t1[:], in0=s1[:], in1=mu[:], op=alu.mult)
            var = small.tile([P, 1], dt.float32, tag="var")
            nc.vector.tensor_tensor(out=var[:], in0=ssq[:], in1=t1[:],
                                    op=alu.subtract)
            std = small.tile([P, 1], dt.float32, tag="std")
            nc.scalar.activation(out=std[:], in_=var[:], func=act_fn.Sqrt,
                                 scale=1.0 / (D - 1))
            spe = small.tile([P, 1], dt.float32, tag="spe")
            nc.vector.tensor_scalar_add(out=spe[:], in0=std[:], scalar1=EPS)
            inv = small.tile([P, 1], dt.float32, tag="inv")
            nc.vector.reciprocal(inv[:], spe[:])

            ms = small.tile([P, 2], dt.float32, tag="ms")
            nc.vector.tensor_copy(ms[:, 0:1], mu[:])
            nc.vector.tensor_copy(ms[:, 1:2], std[:])
            nc.sync.dma_start(out=mustd[r0:r0 + P, :], in_=ms[:])

            nc.vector.tensor_scalar(out=xn[:], in0=xt[:], scalar1=mu[:],
                                    scalar2=inv[:], op0=alu.subtract,
                                    op1=alu.mult)

            # split xn into float32r head + residual
            xr = sb.tile([P, D], dt.float32r, tag="xr")
            nc.vector.tensor_copy(xr[:], xn[:])
            xe = sb.tile([P, D], dt.float32r, tag="xe")
            nc.vector.tensor_tensor(out=xe[:], in0=xn[:],
                                    in1=xr[:].bitcast(dt.float32),
                                    op=alu.subtract)

            # ---- transpose xr/xe -> xrT/xeT (12 PE transposes) ----
            xrT = sb.tile([P, DT, P], dt.float32r, tag="xrT")
            xeT = sb.tile([P, DT, P], dt.float32r, tag="xeT")
            for t in range(DT):
                ptr = pst.tile([P, P], dt.float32, tag="ptr")
                nc.tensor.transpose(ptr[:].bitcast(dt.float32r),
                                    xr[:, t * P:(t + 1) * P], ident[:])
                nc.scalar.copy(xrT[:, t, :], ptr[:])
                pte = pst.tile([P, P], dt.float32, tag="pte")
                nc.tensor.transpose(pte[:].bitcast(dt.float32r),
                                    xe[:, t * P:(t + 1) * P], ident[:])
                nc.scalar.copy(xeT[:, t, :], pte[:])

            # ---- encode (3-term f32r) + chunk candidates ----
            cand = sb.tile([P, NGC * CHUNK_CAND], dt.float32, tag="cand")
            scr = sb.tile([P, 512], dt.float32, tag="scr")
            for gc in range(NGC):
                pe = pse.tile([P, 512], dt.float32, tag="pe")
                for t in range(DT):
                    # xr is stationary for the first two terms (one less
                    # weight reload per step)
                    nc.tensor.matmul(pe[:], xrT[:, t, :],
                                     wr[:, t, gc * 512:(gc + 1) * 512],
                                     start=(t == 0), stop=False)
                    nc.tensor.matmul(pe[:], xrT[:, t, :],
                                     we[:, t, gc * 512:(gc + 1) * 512],
                                     start=False, stop=False)
                    nc.tensor.matmul(pe[:], xeT[:, t, :],
                                     wr[:, t, gc * 512:(gc + 1) * 512],
                                     start=False, stop=(t == DT - 1))
                pchunk = small.tile([P, 512], dt.float32, tag="pchunk")
                nc.scalar.copy(pchunk[:], pe[:])
                nc.sync.dma_start(
                    out=preact[r0:r0 + P, gc * 512:(gc + 1) * 512],
                    in_=pchunk[:])
                c0 = gc * CHUNK_CAND
                nc.vector.max(cand[:, c0:c0 + 8], pchunk[:])
                nc.vector.match_replace(scr[:], cand[:, c0:c0 + 8], pchunk[:],
                                        -3.0e38)
                nc.vector.max(cand[:, c0 + 8:c0 + 16], scr[:])
            nc.sync.dma_start(out=cands[r0:r0 + P, :], in_=cand[:])
    split_waits(nc)
    return nc


def build_l2():
    OH = O // 2             # 1536-wide output half
    nc = bass.Bass(trn_type="TRN2")
    preact = nc.dram_tensor("preact", [B, GS], dt.float32, kind="ExternalInput")
    wdec = nc.dram_tensor("wdec", [GS, O], dt.float32, kind="ExternalInput")
    t64 = nc.dram_tensor("t64", [B, 1], dt.float32, kind="ExternalInput")
    eye = nc.dram_tensor("eye", [P, P], dt.float32, kind="ExternalInput")
    partial = nc.dram_tensor("partial", [B, O], dt.float32, kind="ExternalOutput")
    NGT = GS // P           # 24 g-tiles per slice
    # scratch for feats^T tiles, one contiguous block per row tile:
    # [rt][p, gt, r] with feature g = gt*128 + p, row = rt*128 + r
    ftT_hbm = nc.dram_tensor("ftT_scratch", [NRT, P, NGT, P], dt.float32r)

    with tile.TileContext(nc) as tc, ExitStack() as ctx:
        const = ctx.enter_context(tc.tile_pool(name="const", bufs=1))
        ident = const.tile([P, P], dt.float32, tag="ident")
        nc.sync.dma_start(out=ident[:], in_=eye[:])
        # all 64 per-row-tile threshold columns in one load:
        # t64_sb[p, rt] = t64[rt*128 + p]
        t64_sb = const.tile([P, NRT], dt.float32, tag="t64_sb")
        nc.sync.dma_start(out=t64_sb[:],
                          in_=t64[:].rearrange("(rt p) one -> p (rt one)", p=P))

        OT = O // 3
        wpool = ctx.enter_context(tc.tile_pool(name="wpool", bufs=1))
        sba = ctx.enter_context(tc.tile_pool(name="sba", bufs=2))
        sbm = ctx.enter_context(tc.tile_pool(name="sbm", bufs=1))
        sbt = ctx.enter_context(tc.tile_pool(name="sbt", bufs=2))
        sbb = ctx.enter_context(tc.tile_pool(name="sbb", bufs=2))
        sbo = ctx.enter_context(tc.tile_pool(name="sbo", bufs=3))
        psa = ctx.enter_context(tc.tile_pool(name="psa", bufs=3, space="PSUM"))
        psb = ctx.enter_context(tc.tile_pool(name="psb", bufs=4, space="PSUM"))

        def decode_piece(piece, rt, fT, wd):
            r0 = rt * P
            o0 = piece * OT
            outsb = sbo.tile([P, OT], dt.float32, tag="outsb")
            for oc in range(OT // 512):
                pd = psb.tile([P, 512], dt.float32, tag="pd")
                for gt in range(NGT):
                    nc.tensor.matmul(pd[:], fT[:, gt, :],
                                     wd[:, gt, oc * 512:(oc + 1) * 512],
                                     start=(gt == 0), stop=(gt == NGT - 1))
                nc.scalar.copy(outsb[:, oc * 512:(oc + 1) * 512], pd[:])
            nc.sync.dma_start(out=partial[r0:r0 + P, o0:o0 + OT], in_=outsb[:])

        def load_wd(piece):
            o0 = piece * OT
            wd = wpool.tile([P, NGT, OT], dt.float32r, tag="wd")
            for gt in range(NGT):
                nc.sync.dma_start(
                    out=wd[:, gt, :],
                    in_=wdec[gt * P:(gt + 1) * P,
                             o0:o0 + OT].bitcast(dt.float32r))
            return wd

        # ---- phase A: mask -> feats -> ftT, fused with piece-0 decode ----
        wd = load_wd(0)
        for rt in range(NRT):
            r0 = rt * P
            pt = sba.tile([P, GS], dt.float32, tag="pt")
            nc.sync.dma_start(out=pt[:], in_=preact[r0:r0 + P, :])

            mask = sbm.tile([P, GS], dt.float32, tag="mask")
            nc.vector.tensor_scalar(out=mask[:], in0=pt[:],
                                    scalar1=t64_sb[:, rt:rt + 1],
                                    scalar2=None, op0=alu.is_ge)
            ft = sbm.tile([P, GS], dt.float32, tag="ft")
            nc.vector.scalar_tensor_tensor(out=ft[:], in0=pt[:], scalar=0.0,
                                           in1=mask[:], op0=alu.max,
                                           op1=alu.mult)

            ftT = sbt.tile([P, NGT, P], dt.float32r, tag="ftT")
            for q in range(NGT // 4):
                ptq = psa.tile([P, 512], dt.float32, tag="ptq")
                for j in range(4):
                    gt = q * 4 + j
                    nc.tensor.transpose(ptq[:, j * P:(j + 1) * P],
                                        ft[:, gt * P:(gt + 1) * P], ident[:])
                nc.scalar.copy(ftT[:, q * 4:(q + 1) * 4, :], ptq[:])
            nc.sync.dma_start(out=ftT_hbm[rt], in_=ftT[:])
            decode_piece(0, rt, ftT, wd)

        # ---- pieces 1..2 from the HBM-staged ftT ----
        pieces = () if os.environ.get("L2_PHASE_A_ONLY") else (1, 2)
        for piece in pieces:
            wd = load_wd(piece)
            for rt in range(NRT):
                fT = sbb.tile([P, NGT, P], dt.float32r, tag="fT")
                nc.sync.dma_start(out=fT[:], in_=ftT_hbm[rt])
                decode_piece(piece, rt, fT, wd)
    split_waits(nc)
    return nc


def _round_f32r(a):
    """Round fp32 to float32r precision (11 mantissa bits, round-half-up):
    values become exactly representable, so the PE's own rounding on read
    is then the identity."""
    u = a.view(np.uint32).astype(np.uint64)
    r = ((u + 0x800) & ~np.uint64(0xFFF)).astype(np.uint32)
    return r.view(np.float32)


def kernel(x, W_enc, b_enc, W_dec, b_dec, stats_last_nonzero, topk, auxk,
           dead_steps_threshold):
    x = np.ascontiguousarray(np.asarray(x, dtype=np.float32))
    W_enc = np.ascontiguousarray(np.asarray(W_enc, dtype=np.float32))
    W_dec = np.ascontiguousarray(np.asarray(W_dec, dtype=np.float32))
    b_enc = np.asarray(b_enc, dtype=np.float32)
    b_dec = np.asarray(b_dec, dtype=np.float32)
    stats = np.asarray(stats_last_nonzero)
    k = int(topk)
    ka = int(auxk)
    thr = int(dead_steps_threshold)
    assert x.shape == (B, D) and W_enc.shape == (D, G) and W_dec.shape == (G, O)
    assert k == TOPK, f"kernel compiled for topk=64, got {k}"

    eye = np.eye(P, dtype=np.float32)
    core_ids = list(range(NCORES))

    # ---------------- Launch 1: LN + encode + candidates ----------------
    nc1 = build_l1()
    in_maps1 = []
    for c in core_ids:
        wsl = np.ascontiguousarray(W_enc[:, c * GS:(c + 1) * GS])
        wsl_r = _round_f32r(wsl)
        wsl_e = (wsl - wsl_r).astype(np.float32)
        in_maps1.append({"x": x, "wencr": wsl_r, "wence": wsl_e, "eye": eye})
    r1 = _run_spmd(nc1, in_maps1, core_ids)
    res1 = r1.results

    preacts = [res1[c]["preact"] for c in core_ids]
    cands = np.concatenate([res1[c]["cands"] for c in core_ids], axis=1)
    mustd = res1[0]["mustd"]
    mu = mustd[:, 0:1]
    std = mustd[:, 1:2]

    # b_enc is zero in this problem; fold it in anyway for generality.
    if np.any(b_enc != 0.0):
        for c in core_ids:
            preacts[c] = preacts[c] + b_enc[c * GS:(c + 1) * GS]
        cands = None  # candidates no longer valid; force exact path

    # ---------------- host: exact per-row threshold -------------------
    if cands is not None:
        t64 = np.partition(cands, cands.shape[1] - k, axis=1)[:,
                                                              cands.shape[1] - k]
        chunk16 = cands.reshape(B, -1, CHUNK_CAND)[:, :, CHUNK_CAND - 1]
        flagged = np.nonzero((chunk16 >= t64[:, None]).any(axis=1))[0]
    else:
        flagged = np.arange(B)
        t64 = np.empty(B, dtype=np.float32)
    for r in flagged:
        row = np.concatenate([p[r] for p in preacts])
        t64[r] = np.partition(row, G - k)[G - k]
    t64 = t64.astype(np.float32).reshape(B, 1)

    # ---------------- Launch 2: mask + colsum + decode -----------------
    nc2 = build_l2()
    in_maps2 = [
        {"preact": preacts[c], "t64": t64, "eye": eye,
         "wdec": np.ascontiguousarray(W_dec[c * GS:(c + 1) * GS, :])}
        for c in core_ids
    ]
    r2 = _run_spmd(nc2, in_maps2, core_ids)
    res2 = r2.results

    # feats is reproduced host-side, bit-identically to the device mask:
    # relu(preact) * (preact >= t64), evaluated per slice in fp32.
    feats = np.empty((B, G), dtype=np.float32)
    for c in core_ids:
        p = preacts[c]
        np.multiply(np.maximum(p, 0.0), (p >= t64).astype(np.float32),
                    out=feats[:, c * GS:(c + 1) * GS])
    partial = res2[0]["partial"].copy()
    for c in core_ids[1:]:
        partial += res2[c]["partial"]

    # ---------------- host: fixups, stats, outputs --------------------
    # count/any per feature in one pass over feats (bool view is cheap)
    fired = feats > 0
    counts = fired.sum(axis=1)
    # rows where the mask did not select exactly k strictly-positive
    # features (fp32 ties at the threshold, or a non-positive threshold)
    bad = np.nonzero(counts != k)[0]
    if bad.size:
        rows = np.arange(G)
        for r in bad:
            row = np.concatenate([p[r] for p in preacts])
            order = np.lexsort((rows, -row))[:k]   # ties -> lower index first
            frow = np.zeros(G, dtype=np.float32)
            frow[order] = np.maximum(row[order], 0.0)
            feats[r] = frow
            partial[r] = frow @ W_dec
        fired = feats > 0
    all_zero = ~fired.any(axis=0)

    out = (partial + b_dec) * std + mu

    new_stats = stats * all_zero.astype(stats.dtype) + 1
    dead = new_stats > thr
    n_dead = int(dead.sum())
    if n_dead == 0:
        num_dead = np.float32(0.0)
        dead_recon = np.broadcast_to(b_dec[None, :] * std + mu,
                                     (B, O)).astype(np.float32).copy()
    else:
        # general (slow, host) fallback for the aux path
        didx_all = np.nonzero(dead)[0]
        pre_dead = np.concatenate(
            [preacts[c][:, :] for c in core_ids], axis=1)[:, didx_all]
        auxk_acts_nz = pre_dead          # values at dead features
        num_dead = np.float32(
            (auxk_acts_nz != 0).sum(axis=1).astype(np.float32).mean())
        kk = min(ka, didx_all.size)
        top_dead = np.zeros((B, didx_all.size), dtype=np.float32)
        if didx_all.size > ka:
            part = np.partition(auxk_acts_nz, didx_all.size - ka, axis=1)
            ta_thr = part[:, didx_all.size - ka][:, None]
            sel = auxk_acts_nz >= ta_thr
            top_dead = np.where(sel, auxk_acts_nz, 0.0)
        else:
            top_dead = auxk_acts_nz.copy()
        top_dead = np.maximum(top_dead, 0.0)
        dead_recon = (top_dead @ W_dec[didx_all, :] + b_dec) * std + mu
        dead_recon = dead_recon.astype(np.float32)

    return (feats.astype(np.float32), out.astype(np.float32), dead_recon,
            num_dead)


# revision 42
# speedup vs baseline: 1.0054x; 1.0054x over previous
"""TopK sparse-autoencoder forward on 8 Trainium2 NeuronCores.

Strategy (tensor-parallel over the G=24576 feature dim, 3072 per core):

  Launch 1 (per core): LayerNorm(x) (all rows, replicated work),
    preact slice = xn @ W_enc[:, slice] in exact fp32 on the PE,
    per-512-chunk top-16 candidates per row (vector.max + match_replace),
    preact slice + candidates + (mu, std) -> HBM.

  Host: exact per-row 64th-largest threshold t64 from the merged
    candidates (np.partition), with a sufficiency check per chunk and an
    exact fallback from the full preact slices for any flagged row.

  Launch 2 (per core): feats slice = relu(preact) * (preact >= t64),
    per-feature fire-sums (colsum) for the dead mask, feats^T tiles
    (PE transpose, rounded to float32r) staged via HBM, then the decode
    partial product feats_slice @ W_dec[slice, :] in float32r.

  Host: out = (sum of partials + b_dec) * std + mu; dead mask from the
    colsums (for this problem's data no feature is dead, so the aux path
    degenerates to dead_recon = mu and num_dead = 0; a full numpy
    fallback handles the general case).
"""
import os
import numpy as np
from contextlib import ExitStack

import concourse.bass as bass
import concourse.mybir as mybir
from concourse import tile
from concourse.bass_utils import run_bass_kernel_spmd


def _run_spmd(nc, in_maps, core_ids, tries=3):
    """The axon-tunneled device occasionally hard-faults transiently
    (NRT_EXEC_UNIT_UNRECOVERABLE); retry a couple of times."""
    import time as _time
    last = None
    for i in range(tries):
        try:
            r = run_bass_kernel_spmd(nc, in_maps, core_ids)
            # results can be lazy device arrays; force materialization so
            # transient device faults surface here and get retried
            r.results = [{k: np.asarray(v) for k, v in m.items()}
                         for m in r.results]
            return r
        except Exception as e:   # noqa: BLE001
            last = e
            _time.sleep(5.0 * (i + 1))
    raise last

dt = mybir.dt
alu = mybir.AluOpType
act_fn = mybir.ActivationFunctionType

B, D, G, O = 8192, 768, 24576, 3072
NCORES = 8
GS = G // NCORES            # 3072 features per core
EPS = 1e-5
P = 128
NRT = B // P                # 64 row tiles
DT = D // P                 # 6 d-tiles
NGC = GS // 512             # 6 g-chunks of 512 per core slice
TOPK = 64
CHUNK_CAND = 16             # top-16 kept per 512-chunk


def split_waits(nc, max_waits: int = 1):
    """This walrus build accepts only one sync-wait command per
    instruction; move excess waits onto preceding same-engine NoOps."""
    ctr = 0
    for f in nc.m.functions:
        for b in f.blocks:
            insns = b.instructions
            out = []
            changed = False
            for inst in insns:
                si = inst.sync_info
                if si is not None and si.on_wait and len(si.on_wait) > max_waits:
                    waits = list(si.on_wait)
                    for w in waits[:-max_waits]:
                        ctr += 1
                        nop = mybir.InstNoOp(
                            name=f"wait-split-{ctr}", ins=[], outs=[],
                            engine=inst.engine,
                        )
                        nop.sync_info = mybir.SyncInfo(on_wait=[w], on_update=[])
                        out.append(nop)
                    inst.sync_info = mybir.SyncInfo(
                        on_wait=waits[-max_waits:], on_update=list(si.on_update)
                    )
                    changed = True
                out.append(inst)
            if changed:
                b.instructions = out
    return ctr


def build_l1():
    """LN + exact-precision encode via 3-term float32r split:
    xn @ W ≈ xr@Wr + xe@Wr + xr@We with xr/Wr rounded to float32r and
    xe/We the (rounded) residuals — max abs err ~2e-7, at 3 cycles/row
    instead of fp32's 4."""
    nc = bass.Bass(trn_type="TRN2")
    x = nc.dram_tensor("x", [B, D], dt.float32, kind="ExternalInput")
    wencr = nc.dram_tensor("wencr", [D, GS], dt.float32, kind="ExternalInput")
    wence = nc.dram_tensor("wence", [D, GS], dt.float32, kind="ExternalInput")
    eye = nc.dram_tensor("eye", [P, P], dt.float32, kind="ExternalInput")
    preact = nc.dram_tensor("preact", [B, GS], dt.float32, kind="ExternalOutput")
    cands = nc.dram_tensor("cands", [B, NGC * CHUNK_CAND], dt.float32,
                           kind="ExternalOutput")
    mustd = nc.dram_tensor("mustd", [B, 2], dt.float32, kind="ExternalOutput")

    with tile.TileContext(nc) as tc, ExitStack() as ctx:
        const = ctx.enter_context(tc.tile_pool(name="const", bufs=1))
        wpool = ctx.enter_context(tc.tile_pool(name="wpool", bufs=1))
        sb = ctx.enter_context(tc.tile_pool(name="sb", bufs=2))
        small = ctx.enter_context(tc.tile_pool(name="small", bufs=4))
        pst = ctx.enter_context(tc.tile_pool(name="pst", bufs=2, space="PSUM"))
        pse = ctx.enter_context(tc.tile_pool(name="pse", bufs=3, space="PSUM"))

        ident = const.tile([P, P], dt.float32r, tag="ident")
        nc.sync.dma_start(out=ident[:], in_=eye[:].bitcast(dt.float32r))

        # resident W_enc slice splits: block t holds rows [t*128,(t+1)*128)
        wr = wpool.tile([P, DT, GS], dt.float32r, tag="wr")
        we = wpool.tile([P, DT, GS], dt.float32r, tag="we")
        for t in range(DT):
            nc.sync.dma_start(out=wr[:, t, :],
                              in_=wencr[t * P:(t + 1) * P, :].bitcast(dt.float32r))
            nc.sync.dma_start(out=we[:, t, :],
                              in_=wence[t * P:(t + 1) * P, :].bitcast(dt.float32r))

        for rt in range(NRT):
            r0 = rt * P
            xt = sb.tile([P, D], dt.float32, tag="xt")
            nc.sync.dma_start(out=xt[:], in_=x[r0:r0 + P, :])

            # ---- LayerNorm stats ----
            s1 = small.tile([P, 1], dt.float32, tag="s1")
            nc.vector.tensor_reduce(out=s1[:], in_=xt[:],
                                    axis=mybir.AxisListType.XYZW, op=alu.add)
            xn = sb.tile([P, D], dt.float32, tag="xn")
            ssq = small.tile([P, 1], dt.float32, tag="ssq")
            # squares land in xn's buffer (overwritten right after)
            nc.scalar.activation(out=xn[:], in_=xt[:], func=act_fn.Square,
                                 accum_out=ssq[:])
            mu = small.tile([P, 1], dt.float32, tag="mu")
            nc.vector.tensor_scalar_mul(out=mu[:], in0=s1[:], scalar1=1.0 / D)
            t1 = small.tile([P, 1], dt.float32, tag="t1")
            nc.vector.tensor_tensor(out=t1[:], in0=s1[:], in1=mu[:], op=alu.mult)
            var = small.tile([P, 1], dt.float32, tag="var")
            nc.vector.tensor_tensor(out=var[:], in0=ssq[:], in1=t1[:],
                                    op=alu.subtract)
            std = small.tile([P, 1], dt.float32, tag="std")
            nc.scalar.activation(out=std[:], in_=var[:], func=act_fn.Sqrt,
                                 scale=1.0 / (D - 1))
            spe = small.tile([P, 1], dt.float32, tag="spe")
            nc.vector.tensor_scalar_add(out=spe[:], in0=std[:], scalar1=EPS)
            inv = small.tile([P, 1], dt.float32, tag="inv")
            nc.vector.reciprocal(inv[:], spe[:])

            ms = small.tile([P, 2], dt.float32, tag="ms")
            nc.vector.tensor_copy(ms[:, 0:1], mu[:])
            nc.vector.tensor_copy(ms[:, 1:2], std[:])
            nc.sync.dma_start(out=mustd[r0:r0 + P, :], in_=ms[:])

            nc.vector.tensor_scalar(out=xn[:], in0=xt[:], scalar1=mu[:],
                                    scalar2=inv[:], op0=alu.subtract,
                                    op1=alu.mult)

            # split xn into float32r head + residual
            xr = sb.tile([P, D], dt.float32r, tag="xr")
            nc.vector.tensor_copy(xr[:], xn[:])
            xe = sb.tile([P, D], dt.float32r, tag="xe")
            nc.vector.tensor_tensor(out=xe[:], in0=xn[:],
                                    in1=xr[:].bitcast(dt.float32),
                                    op=alu.subtract)

            # ---- transpose xr/xe -> xrT/xeT (12 PE transposes) ----
            xrT = sb.tile([P, DT, P], dt.float32r, tag="xrT")
            xeT = sb.tile([P, DT, P], dt.float32r, tag="xeT")
            for t in range(DT):
                ptr = pst.tile([P, P], dt.float32, tag="ptr")
                nc.tensor.transpose(ptr[:].bitcast(dt.float32r),
                                    xr[:, t * P:(t + 1) * P], ident[:])
                nc.scalar.copy(xrT[:, t, :], ptr[:])
                pte = pst.tile([P, P], dt.float32, tag="pte")
                nc.tensor.transpose(pte[:].bitcast(dt.float32r),
                                    xe[:, t * P:(t + 1) * P], ident[:])
                nc.scalar.copy(xeT[:, t, :], pte[:])

            # ---- encode (3-term f32r) + chunk candidates ----
            cand = sb.tile([P, NGC * CHUNK_CAND], dt.float32, tag="cand")
            scr = sb.tile([P, 512], dt.float32, tag="scr")
            for gc in range(NGC):
                pe = pse.tile([P, 512], dt.float32, tag="pe")
                for t in range(DT):
                    # xr is stationary for the first two terms (one less
                    # weight reload per step)
                    nc.tensor.matmul(pe[:], xrT[:, t, :],
                                     wr[:, t, gc * 512:(gc + 1) * 512],
                                     start=(t == 0), stop=False)
                    nc.tensor.matmul(pe[:], xrT[:, t, :],
                                     we[:, t, gc * 512:(gc + 1) * 512],
                                     start=False, stop=False)
                    nc.tensor.matmul(pe[:], xeT[:, t, :],
                                     wr[:, t, gc * 512:(gc + 1) * 512],
                                     start=False, stop=(t == DT - 1))
                pchunk = small.tile([P, 512], dt.float32, tag="pchunk")
                nc.scalar.copy(pchunk[:], pe[:])
                nc.sync.dma_start(
                    out=preact[r0:r0 + P, gc * 512:(gc + 1) * 512],
                    in_=pchunk[:])
                c0 = gc * CHUNK_CAND
                nc.vector.max(cand[:, c0:c0 + 8], pchunk[:])
                nc.vector.match_replace(scr[:], cand[:, c0:c0 + 8], pchunk[:],
                                        -3.0e38)
                nc.vector.max(cand[:, c0 + 8:c0 + 16], scr[:])
            nc.sync.dma_start(out=cands[r0:r0 + P, :], in_=cand[:])
    split_waits(nc)
    return nc


def build_l2():
    OH = O // 2             # 1536-wide output half
    nc = bass.Bass(trn_type="TRN2")
    preact = nc.dram_tensor("preact", [B, GS], dt.float32, kind="ExternalInput")
    wdec = nc.dram_tensor("wdec", [GS, O], dt.float32, kind="ExternalInput")
    t64 = nc.dram_tensor("t64", [B, 1], dt.float32, kind="ExternalInput")
    eye = nc.dram_tensor("eye", [P, P], dt.float32, kind="ExternalInput")
    partial = nc.dram_tensor("partial", [B, O], dt.float32, kind="ExternalOutput")
    NGT = GS // P           # 24 g-tiles per slice
    # scratch for feats^T tiles, one contiguous block per row tile:
    # [rt][p, gt, r] with feature g = gt*128 + p, row = rt*128 + r
    ftT_hbm = nc.dram_tensor("ftT_scratch", [NRT, P, NGT, P], dt.float32r)

    with tile.TileContext(nc) as tc, ExitStack() as ctx:
        const = ctx.enter_context(tc.tile_pool(name="const", bufs=1))
        ident = const.tile([P, P], dt.float32r, tag="ident")
        nc.sync.dma_start(out=ident[:], in_=eye[:].bitcast(dt.float32r))
        # all 64 per-row-tile threshold columns in one load:
        # t64_sb[p, rt] = t64[rt*128 + p]
        t64_sb = const.tile([P, NRT], dt.float32, tag="t64_sb")
        nc.sync.dma_start(out=t64_sb[:],
                          in_=t64[:].rearrange("(rt p) one -> p (rt one)", p=P))

        OT = O // 3
        wpool = ctx.enter_context(tc.tile_pool(name="wpool", bufs=1))
        sba = ctx.enter_context(tc.tile_pool(name="sba", bufs=2))
        sbm = ctx.enter_context(tc.tile_pool(name="sbm", bufs=1))
        sbt = ctx.enter_context(tc.tile_pool(name="sbt", bufs=2))
        sbb = ctx.enter_context(tc.tile_pool(name="sbb", bufs=2))
        sbo = ctx.enter_context(tc.tile_pool(name="sbo", bufs=3))
        psa = ctx.enter_context(tc.tile_pool(name="psa", bufs=3, space="PSUM"))
        psb = ctx.enter_context(tc.tile_pool(name="psb", bufs=4, space="PSUM"))

        def decode_piece(piece, rt, fT, wd):
            r0 = rt * P
            o0 = piece * OT
            outsb = sbo.tile([P, OT], dt.float32, tag="outsb")
            for oc in range(OT // 512):
                pd = psb.tile([P, 512], dt.float32, tag="pd")
                for gt in range(NGT):
                    nc.tensor.matmul(pd[:], fT[:, gt, :],
                                     wd[:, gt, oc * 512:(oc + 1) * 512],
                                     start=(gt == 0), stop=(gt == NGT - 1))
                nc.scalar.copy(outsb[:, oc * 512:(oc + 1) * 512], pd[:])
            nc.sync.dma_start(out=partial[r0:r0 + P, o0:o0 + OT], in_=outsb[:])

        def load_wd(piece):
            o0 = piece * OT
            wd = wpool.tile([P, NGT, OT], dt.float32r, tag="wd")
            for gt in range(NGT):
                nc.sync.dma_start(
                    out=wd[:, gt, :],
                    in_=wdec[gt * P:(gt + 1) * P,
                             o0:o0 + OT].bitcast(dt.float32r))
            return wd

        # ---- phase A: mask -> feats -> ftT, fused with piece-0 decode ----
        wd = load_wd(0)
        for rt in range(NRT):
            r0 = rt * P
            pt = sba.tile([P, GS], dt.float32, tag="pt")
            nc.sync.dma_start(out=pt[:], in_=preact[r0:r0 + P, :])

            mask = sbm.tile([P, GS], dt.float32, tag="mask")
            nc.vector.tensor_scalar(out=mask[:], in0=pt[:],
                                    scalar1=t64_sb[:, rt:rt + 1],
                                    scalar2=None, op0=alu.is_ge)
            # ft in float32r: the decode path rounds to f32r anyway, and
            # f32r PE transposes run at 1.5 cyc/row instead of fp32's 2
            ft = sbm.tile([P, GS], dt.float32r, tag="ft")
            nc.vector.scalar_tensor_tensor(out=ft[:], in0=pt[:], scalar=0.0,
                                           in1=mask[:], op0=alu.max,
                                           op1=alu.mult)

            ftT = sbt.tile([P, NGT, P], dt.float32r, tag="ftT")
            for q in range(NGT // 4):
                ptq = psa.tile([P, 512], dt.float32, tag="ptq")
                for j in range(4):
                    gt = q * 4 + j
                    nc.tensor.transpose(
                        ptq[:, j * P:(j + 1) * P].bitcast(dt.float32r),
                        ft[:, gt * P:(gt + 1) * P], ident[:])
                nc.scalar.copy(ftT[:, q * 4:(q + 1) * 4, :], ptq[:])
            nc.sync.dma_start(out=ftT_hbm[rt], in_=ftT[:])
            decode_piece(0, rt, ftT, wd)

        # ---- pieces 1..2 from the HBM-staged ftT ----
        pieces = () if os.environ.get("L2_PHASE_A_ONLY") else (1, 2)
        for piece in pieces:
            wd = load_wd(piece)
            for rt in range(NRT):
                fT = sbb.tile([P, NGT, P], dt.float32r, tag="fT")
                nc.sync.dma_start(out=fT[:], in_=ftT_hbm[rt])
                decode_piece(piece, rt, fT, wd)
    split_waits(nc)
    return nc


def _round_f32r(a):
    """Round fp32 to float32r precision (11 mantissa bits, round-half-up):
    values become exactly representable, so the PE's own rounding on read
    is then the identity."""
    u = a.view(np.uint32).astype(np.uint64)
    r = ((u + 0x800) & ~np.uint64(0xFFF)).astype(np.uint32)
    return r.view(np.float32)


def kernel(x, W_enc, b_enc, W_dec, b_dec, stats_last_nonzero, topk, auxk,
           dead_steps_threshold):
    x = np.ascontiguousarray(np.asarray(x, dtype=np.float32))
    W_enc = np.ascontiguousarray(np.asarray(W_enc, dtype=np.float32))
    W_dec = np.ascontiguousarray(np.asarray(W_dec, dtype=np.float32))
    b_enc = np.asarray(b_enc, dtype=np.float32)
    b_dec = np.asarray(b_dec, dtype=np.float32)
    stats = np.asarray(stats_last_nonzero)
    k = int(topk)
    ka = int(auxk)
    thr = int(dead_steps_threshold)
    assert x.shape == (B, D) and W_enc.shape == (D, G) and W_dec.shape == (G, O)
    assert k == TOPK, f"kernel compiled for topk=64, got {k}"

    eye = np.eye(P, dtype=np.float32)
    core_ids = list(range(NCORES))

    # ---------------- Launch 1: LN + encode + candidates ----------------
    nc1 = build_l1()
    in_maps1 = []
    for c in core_ids:
        wsl = np.ascontiguousarray(W_enc[:, c * GS:(c + 1) * GS])
        wsl_r = _round_f32r(wsl)
        wsl_e = (wsl - wsl_r).astype(np.float32)
        in_maps1.append({"x": x, "wencr": wsl_r, "wence": wsl_e, "eye": eye})
    r1 = _run_spmd(nc1, in_maps1, core_ids)
    res1 = r1.results

    preacts = [res1[c]["preact"] for c in core_ids]
    cands = np.concatenate([res1[c]["cands"] for c in core_ids], axis=1)
    mustd = res1[0]["mustd"]
    mu = mustd[:, 0:1]
    std = mustd[:, 1:2]

    # b_enc is zero in this problem; fold it in anyway for generality.
    if np.any(b_enc != 0.0):
        for c in core_ids:
            preacts[c] = preacts[c] + b_enc[c * GS:(c + 1) * GS]
        cands = None  # candidates no longer valid; force exact path

    # ---------------- host: exact per-row threshold -------------------
    if cands is not None:
        t64 = np.partition(cands, cands.shape[1] - k, axis=1)[:,
                                                              cands.shape[1] - k]
        chunk16 = cands.reshape(B, -1, CHUNK_CAND)[:, :, CHUNK_CAND - 1]
        flagged = np.nonzero((chunk16 >= t64[:, None]).any(axis=1))[0]
    else:
        flagged = np.arange(B)
        t64 = np.empty(B, dtype=np.float32)
    for r in flagged:
        row = np.concatenate([p[r] for p in preacts])
        t64[r] = np.partition(row, G - k)[G - k]
    t64 = t64.astype(np.float32).reshape(B, 1)

    # ---------------- Launch 2: mask + colsum + decode -----------------
    nc2 = build_l2()
    in_maps2 = [
        {"preact": preacts[c], "t64": t64, "eye": eye,
         "wdec": np.ascontiguousarray(W_dec[c * GS:(c + 1) * GS, :])}
        for c in core_ids
    ]
    r2 = _run_spmd(nc2, in_maps2, core_ids)
    res2 = r2.results

    # feats is reproduced host-side, bit-identically to the device mask:
    # relu(preact) * (preact >= t64), evaluated per slice in fp32.
    feats = np.empty((B, G), dtype=np.float32)
    for c in core_ids:
        p = preacts[c]
        np.multiply(np.maximum(p, 0.0), (p >= t64).astype(np.float32),
                    out=feats[:, c * GS:(c + 1) * GS])
    partial = res2[0]["partial"].copy()
    for c in core_ids[1:]:
        partial += res2[c]["partial"]

    # ---------------- host: fixups, stats, outputs --------------------
    # count/any per feature in one pass over feats (bool view is cheap)
    fired = feats > 0
    counts = fired.sum(axis=1)
    # rows where the mask did not select exactly k strictly-positive
    # features (fp32 ties at the threshold, or a non-positive threshold)
    bad = np.nonzero(counts != k)[0]
    if bad.size:
        rows = np.arange(G)
        for r in bad:
            row = np.concatenate([p[r] for p in preacts])
            order = np.lexsort((rows, -row))[:k]   # ties -> lower index first
            frow = np.zeros(G, dtype=np.float32)
            frow[order] = np.maximum(row[order], 0.0)
            feats[r] = frow
            partial[r] = frow @ W_dec
        fired = feats > 0
    all_zero = ~fired.any(axis=0)

    out = (partial + b_dec) * std + mu

    new_stats = stats * all_zero.astype(stats.dtype) + 1
    dead = new_stats > thr
    n_dead = int(dead.sum())
    if n_dead == 0:
        num_dead = np.float32(0.0)
        dead_recon = np.broadcast_to(b_dec[None, :] * std + mu,
                                     (B, O)).astype(np.float32).copy()
    else:
        # general (slow, host) fallback for the aux path
        didx_all = np.nonzero(dead)[0]
        pre_dead = np.concatenate(
            [preacts[c][:, :] for c in core_ids], axis=1)[:, didx_all]
        auxk_acts_nz = pre_dead          # values at dead features
        num_dead = np.float32(
            (auxk_acts_nz != 0).sum(axis=1).astype(np.float32).mean())
        kk = min(ka, didx_all.size)
        top_dead = np.zeros((B, didx_all.size), dtype=np.float32)
        if didx_all.size > ka:
            part = np.partition(auxk_acts_nz, didx_all.size - ka, axis=1)
            ta_thr = part[:, didx_all.size - ka][:, None]
            sel = auxk_acts_nz >= ta_thr
            top_dead = np.where(sel, auxk_acts_nz, 0.0)
        else:
            top_dead = auxk_acts_nz.copy()
        top_dead = np.maximum(top_dead, 0.0)
        dead_recon = (top_dead @ W_dec[didx_all, :] + b_dec) * std + mu
        dead_recon = dead_recon.astype(np.float32)

    return (feats.astype(np.float32), out.astype(np.float32), dead_recon,
            num_dead)
